# revision 20
# baseline (speedup 1.0000x reference)
"""Trainium2 Bass kernel for DisplaceChannel — conv-then-shift design.

Math (per channel c, group f = c // 16):
  off_px = offset[f] * 64; off_int = round(off_px); sub = off_px - off_int
  out = shift(depthwise 3x3 SAME conv of the window-masked image), where
  the window is the set of source pixels that survive the shift.  Key
  identity: shifting then convolving (reference order) equals convolving
  the *unshifted* window-masked image and shifting the result, because
  both reduce to the same zero-outside-window source semantics.

This removes all shift geometry from the compute and the DMA inner
dims:
  - Loads are per-group contiguous row bands of full-width rows
    (16 descriptors per group instead of ~16*rows).
  - Compute is a uniform separable 3-tap conv over [128, band, 64]
    tiles in bf16 (DVE 2-src ops run 2x at 16 bit).
  - Stores exploit flat addressing: the x-shift becomes a constant
    element offset into the flattened [rows, 64] plane, so each channel
    stores one contiguous run (16 descriptors per group).  Column wrap
    garbage lands only in output columns outside the valid window,
    which the host overwrites with zeros during reassembly.

Row masking: each group loads exactly its valid source rows into a
persistent per-block S tile whose margin rows are zeroed once at
startup and never rewritten.  Column masking: only the one column
adjacent to the window edge can pollute a valid output (conv reach is
1), so it is re-zeroed in Sb after each cast (small DVE memsets).

Engines: sync = HWDGE loads, gpsimd = cast fp32->bf16 + SWDGE
bf16->fp32 cast stores, ACT = center taps, DVE = four side taps +
boundary-column memsets.  Batch-parallel over 8 cores (2 batches per
core), identical SPMD program.
"""

import os
import sys
from collections import deque
from contextlib import ExitStack

import numpy as np

for _p in ("/opt/trn_rl_repo", "/root/.axon_site/_ro/trn_rl_repo"):
    if os.path.isdir(_p) and _p not in sys.path:
        sys.path.append(_p)

import concourse.bass as bass
import concourse.bacc as bacc
import concourse.mybir as mybir
import concourse.tile as tile
from concourse.bass_utils import run_bass_kernel_spmd

H = W = 64
C = 768
B = 16
N_CORES = 8
BPC = B // N_CORES          # batches per core
P = 128                     # partitions
NGRP = 48
GSZ = 16                    # channels per group
SCALE = 64.0
SIGMA = 0.5
FP32 = mybir.dt.float32
BF16 = mybir.dt.bfloat16
MULT = mybir.AluOpType.mult
ADD = mybir.AluOpType.add


def _geometry(offset: np.ndarray):
    """Integer shifts and separable 1-D taps per group, matching reference."""
    off_px = offset.astype(np.float32) * np.float32(SCALE)
    off_int = np.round(off_px)
    sub = off_px - off_int                      # [48, 2] (x, y)
    dx = off_int[:, 0].astype(np.int64)
    dy = off_int[:, 1].astype(np.int64)
    r = (np.arange(3, dtype=np.float32) - 1.0).astype(np.float32)
    ex = np.exp(-((r[None, :] + sub[:, 0:1]) ** 2) / (2.0 * SIGMA * SIGMA))
    ey = np.exp(-((r[None, :] + sub[:, 1:2]) ** 2) / (2.0 * SIGMA * SIGMA))
    v = ex / ex.sum(1, keepdims=True)           # [48, 3] horizontal taps
    u = ey / ey.sum(1, keepdims=True)           # [48, 3] vertical taps
    return dx, dy, v.astype(np.float32), u.astype(np.float32)


class _Geo:
    """Per-group window geometry + block partition."""

    def __init__(self, offset: np.ndarray):
        self.dx, self.dy, self.v, self.u = _geometry(offset)
        self.ry0 = np.maximum(0, -self.dy)
        self.ry1 = H - np.maximum(0, self.dy)
        self.cx0 = np.maximum(0, -self.dx)
        self.cx1 = W - np.maximum(0, self.dx)
        self.rows = self.ry1 - self.ry0          # valid source rows
        # valid dst windows (host reassembly)
        self.vy0 = np.maximum(0, self.ry0 - 1 + self.dy)
        self.vy1 = np.minimum(H, self.ry1 + 1 + self.dy)
        self.vx0 = np.maximum(0, self.cx0 - 1 + self.dx)
        self.vx1 = np.minimum(W, self.cx1 + 1 + self.dx)
        self.blocks = self._partition()

    def _partition(self):
        """Sort groups by band desc, split into consecutive runs of <=8
        minimizing sum of (block band + fixed overhead)."""
        order = sorted(range(NGRP), key=lambda g: -int(self.rows[g]))
        INF = float("inf")
        best = [INF] * (NGRP + 1)
        prev = [0] * (NGRP + 1)
        best[0] = 0.0
        for e in range(1, NGRP + 1):
            for s in range(max(0, e - 8), e):
                band = max(int(self.rows[order[i]]) for i in range(s, e))
                cost = best[s] + band + 10.0
                if cost < best[e]:
                    best[e] = cost
                    prev[e] = s
        cuts = []
        e = NGRP
        while e > 0:
            s = prev[e]
            cuts.append((s, e))
            e = s
        blocks = []
        for s, e in reversed(cuts):
            groups = [order[i] for i in range(s, e)]
            bnd = max(int(self.rows[g]) for g in groups)
            # distinct out-of-window boundary columns to zero in Sb
            badcols = sorted({
                (-int(self.dx[g]) - 1) if self.dx[g] < 0 else (W - int(self.dx[g]))
                for g in groups if self.dx[g] != 0})
            blocks.append((groups, bnd, badcols))
        return blocks


def _build(offset: np.ndarray) -> bass.Bass:
    geo = _Geo(offset)
    nblk = len(geo.blocks)

    # Per-block tap tables.  All compute ops run on the full 128
    # partitions (the ISA requires 32-aligned partition offsets), so
    # group-conditional ops use per-partition scalars that are zero (or
    # one, for the keep multiplies) on non-member partitions.
    # wc fp32 [P, 4]: v1, u1 (center taps), u0neg, u2pos (widening
    # corners).  ws fp32->bf16 [P, 16]: v0, v2, u0, u2 (side taps),
    # v0neg, v2pos (widening T taps), keep_neg, keep_pos, then up to 8
    # per-partition masks for the boundary-column zeroing.
    # single fp32 table [P, 20] per block:
    # 0:v1 1:v0 2:v2 3:u1 4:u0 5:u2 6:v0neg 7:v2pos 8:keep_neg
    # 9:keep_pos 10:u0neg 11:u2pos 12..19: boundary-column masks
    NW = 20
    ws = np.zeros((nblk, P, NW), dtype=np.float32)
    for bi, (groups, _, badcols) in enumerate(geo.blocks):
        assert len(badcols) <= 8
        for sl, g in enumerate(groups):
            pp = slice(sl * GSZ, (sl + 1) * GSZ)
            dxg = int(geo.dx[g])
            ws[bi, pp, 0] = geo.v[g, 1]
            ws[bi, pp, 1] = geo.v[g, 0]
            ws[bi, pp, 2] = geo.v[g, 2]
            ws[bi, pp, 3] = geo.u[g, 1]
            ws[bi, pp, 4] = geo.u[g, 0]
            ws[bi, pp, 5] = geo.u[g, 2]
            ws[bi, pp, 6] = geo.v[g, 0] if dxg < 0 else 0.0
            ws[bi, pp, 7] = geo.v[g, 2] if dxg > 0 else 0.0
            ws[bi, pp, 8] = 0.0 if dxg < 0 else 1.0
            ws[bi, pp, 9] = 0.0 if dxg > 0 else 1.0
            ws[bi, pp, 10] = geo.u[g, 0] if dxg < 0 else 0.0
            ws[bi, pp, 11] = geo.u[g, 2] if dxg > 0 else 0.0
            mybad = (-dxg - 1) if dxg < 0 else (W - dxg)
            for j, c in enumerate(badcols):
                ws[bi, pp, 12 + j] = 0.0 if (dxg != 0 and mybad == c) else 1.0

    nc = bacc.Bacc("TRN2", target_bir_lowering=False, debug=False)
    x_in = nc.dram_tensor("x", [BPC, C, H, W], FP32, kind="ExternalInput")
    y_out = nc.dram_tensor("y", [BPC, C, H, W], FP32, kind="ExternalOutput")
    ws_dram = nc.inline_tensor(ws, name="tapss")

    with tile.TileContext(nc) as tc, ExitStack() as ctx:
        w_pool = ctx.enter_context(tc.tile_pool(name="w", bufs=1))
        s_pool = ctx.enter_context(tc.tile_pool(name="s", bufs=1))
        o_pool = ctx.enter_context(tc.tile_pool(name="o", bufs=3))

        wsb = []
        for bi in range(nblk):
            f = w_pool.tile([P, NW], FP32, name=f"wf{bi}", tag=f"wf{bi}")
            nc.sync.dma_start(f[:], ws_dram[bi])
            wsb.append(f)
        t_pool = ctx.enter_context(tc.tile_pool(name="tmp", bufs=2))

        # Persistent per-block tiles.  S holds the loaded fp32 source
        # rows at local rows [1, 1+rows_g) with zero margins; Sb is its
        # bf16 copy; T is the horizontal-pass result with one extra
        # zero margin row on each side.
        St, Tt = [], []
        for bi, (groups, bnd, _) in enumerate(geo.blocks):
            S = s_pool.tile([P, bnd + 2, W], FP32, name=f"S{bi}", tag=f"S{bi}")
            T = s_pool.tile([P, bnd + 4, W], BF16, name=f"T{bi}", tag=f"T{bi}")
            St.append(S)
            Tt.append(T)

        inited = set()

        def emit_init(bi):
            # zero-margin invariants, established once per block (loads
            # rewrite only each group's interior rows afterwards).
            # Spread across engines so startup ramps fast.
            groups, bnd, _ = geo.blocks[bi]
            S, T = St[bi], Tt[bi]
            eng = (nc.vector, nc.gpsimd, nc.scalar)[bi % 3]
            if eng is nc.scalar:
                eng.memzero(S[:, :, :])
            else:
                eng.memset(S[:, :, :], 0.0)
            nc.gpsimd.memset(T[:, 0:bnd + 4:bnd + 3, :], 0.0)
            inited.add(bi)

        def emit_front(b, bi):
            if bi not in inited:
                emit_init(bi)
            groups, bnd, badcols = geo.blocks[bi]
            S, T = St[bi], Tt[bi]
            w = wsb[bi]
            nb2 = bnd + 2
            f = nb2 * W
            for sl, g in enumerate(groups):
                rg = int(geo.rows[g])
                nc.sync.dma_start(
                    S[sl * GSZ:(sl + 1) * GSZ, 1:1 + rg, :],
                    x_in[b, g * GSZ:(g + 1) * GSZ,
                         int(geo.ry0[g]):int(geo.ry1[g]), :])
            # zero the single source column adjacent to each window edge
            # (the only out-of-window column the conv can reach) via
            # per-partition 0/1 masks, directly on fp32 S.
            for j, c in enumerate(badcols):
                nc.vector.tensor_scalar_mul(
                    S[:, 0:nb2, c:c + 1], S[:, 0:nb2, c:c + 1],
                    w[:, 12 + j:13 + j])
            # h-pass: ACT computes the three per-partition-scaled copies
            # (fp32 -> bf16), DVE combines with 2x tensor_tensor adds.
            Sf = S.rearrange("p r c -> p (r c)")
            Tf = T.rearrange("p r c -> p (r c)")
            Ta = t_pool.tile([P, nb2 * W], BF16, name="Ta", tag="Ta")
            Tb = t_pool.tile([P, nb2 * W], BF16, name="Tb", tag="Tb")
            nc.scalar.mul(Tf[:, W:W + f], Sf[:, 0:f], w[:, 0:1])
            nc.scalar.mul(Ta[:, 0:f], Sf[:, 0:f], w[:, 1:2])
            nc.scalar.mul(Tb[:, 0:f], Sf[:, 0:f], w[:, 2:3])
            nc.vector.tensor_tensor(
                T[:, 1:1 + nb2, 1:W], Ta.rearrange(
                    "p (r c) -> p r c", c=W)[:, 0:nb2, 0:W - 1],
                T[:, 1:1 + nb2, 1:W], ADD)
            nc.vector.tensor_tensor(
                T[:, 1:1 + nb2, 0:W - 1], Tb.rearrange(
                    "p (r c) -> p r c", c=W)[:, 0:nb2, 1:W],
                T[:, 1:1 + nb2, 0:W - 1], ADD)
            # widening columns (same masked keep/accumulate scheme)
            nc.vector.tensor_scalar_mul(
                T[:, 1:2 + nb2, 0:1], T[:, 1:2 + nb2, 0:1], w[:, 8:9])
            nc.vector.scalar_tensor_tensor(
                T[:, 2:2 + nb2, 0:1], S[:, 0:nb2, W - 1:W], w[:, 6:7],
                T[:, 2:2 + nb2, 0:1], MULT, ADD)
            nc.vector.tensor_scalar_mul(
                T[:, 0:1 + nb2, W - 1:W], T[:, 0:1 + nb2, W - 1:W],
                w[:, 9:10])
            nc.vector.scalar_tensor_tensor(
                T[:, 0:nb2, W - 1:W], S[:, 0:nb2, 0:1], w[:, 7:8],
                T[:, 0:nb2, W - 1:W], MULT, ADD)

        def emit_back(b, bi):
            groups, bnd, badcols = geo.blocks[bi]
            T = Tt[bi]
            w = wsb[bi]
            nb2 = bnd + 2
            f = nb2 * W
            Tf = T.rearrange("p r c -> p (r c)")
            O = o_pool.tile([P, bnd + 4, W], BF16, name="O", tag="O")
            nc.gpsimd.memset(O[:, 0:bnd + 4:bnd + 3, :], 0.0)
            Of = O.rearrange("p r c -> p (r c)")
            Oa = t_pool.tile([P, nb2 * W], BF16, name="Oa", tag="Oa")
            Ob = t_pool.tile([P, nb2 * W], BF16, name="Ob", tag="Ob")
            # v-pass: bf16 tensor_scalar premuls (4x) + tensor_tensor
            # adds (2x), all flat contiguous
            # widening-column corners first so ACT never waits on the
            # DVE v-chain (they touch only the edge rows)
            nc.scalar.mul(O[:, bnd + 3:bnd + 4, 0:1],
                          T[:, bnd + 2:bnd + 3, 0:1], w[:, 10:11])
            nc.scalar.mul(O[:, 0:1, W - 1:W],
                          T[:, 1:2, W - 1:W], w[:, 11:12])
            nc.vector.tensor_scalar_mul(Of[:, W:W + f], Tf[:, W:W + f],
                                        w[:, 3:4])
            nc.vector.tensor_scalar_mul(Oa[:, 0:f], Tf[:, 0:f], w[:, 4:5])
            nc.vector.tensor_tensor(Of[:, W:W + f], Oa[:, 0:f],
                                    Of[:, W:W + f], ADD)
            nc.vector.tensor_scalar_mul(Ob[:, 0:f], Tf[:, 2 * W:2 * W + f],
                                        w[:, 5:6])
            nc.vector.tensor_tensor(Of[:, W:W + f], Ob[:, 0:f],
                                    Of[:, W:W + f], ADD)
            for sl, g in enumerate(groups):
                vy0, vy1 = int(geo.vy0[g]), int(geo.vy1[g])
                f0 = (vy0 - int(geo.dy[g]) - int(geo.ry0[g]) + 2) * W \
                    - int(geo.dx[g])
                ln = (vy1 - vy0) * W
                nc.gpsimd.dma_start(
                    y_out[b, g * GSZ:(g + 1) * GSZ, vy0:vy1, :],
                    Of[sl * GSZ:(sl + 1) * GSZ, f0:f0 + ln])

        tiles = [(b, bi) for b in range(BPC) for bi in range(nblk)]
        pend = deque()
        DEPTH = 2
        for b, bi in tiles:
            emit_front(b, bi)
            pend.append((b, bi))
            if len(pend) > DEPTH:
                emit_back(*pend.popleft())
        while pend:
            emit_back(*pend.popleft())

    nc.compile()
    return nc


def _assemble(geo: _Geo, parts: list[np.ndarray]) -> np.ndarray:
    """Gather per-core device outputs into the full zero-padded result."""
    out = np.zeros((B, C, H, W), dtype=np.float32)
    for k, yk in enumerate(parts):
        for b in range(BPC):
            bb = k * BPC + b
            for g in range(NGRP):
                ch = slice(g * GSZ, (g + 1) * GSZ)
                vy0, vy1 = int(geo.vy0[g]), int(geo.vy1[g])
                vx0, vx1 = int(geo.vx0[g]), int(geo.vx1[g])
                out[bb, ch, vy0:vy1, vx0:vx1] = \
                    yk[b, ch, vy0:vy1, vx0:vx1]
    return out


def _run(x: np.ndarray, offset: np.ndarray, trace: bool = False):
    x = np.ascontiguousarray(x, dtype=np.float32)
    offset = np.ascontiguousarray(offset, dtype=np.float32)
    geo = _Geo(offset)
    nc = _build(offset)
    in_maps = [
        {"x": x[k * BPC:(k + 1) * BPC]} for k in range(N_CORES)
    ]
    res = run_bass_kernel_spmd(
        nc, in_maps, core_ids=list(range(N_CORES)), trace=trace
    )
    out = _assemble(geo, [res.results[k]["y"] for k in range(N_CORES)])
    return out, res


def kernel(x: np.ndarray, offset: np.ndarray) -> np.ndarray:
    return _run(x, offset)[0]


def _numpy_sim(x: np.ndarray, offset: np.ndarray) -> np.ndarray:
    """Pure-numpy emulation of the exact device dataflow (fp32, no bf16
    rounding) for index validation."""
    geo = _Geo(offset)
    out_parts = []
    for k in range(N_CORES):
        xb = x[k * BPC:(k + 1) * BPC]
        ydev = np.full((BPC, C, H, W), np.nan, dtype=np.float32)
        Sts = {}
        for bi, (groups, bnd, _) in enumerate(geo.blocks):
            Sts[bi] = np.zeros((P, bnd + 2, W), dtype=np.float32)
        for b in range(BPC):
            for bi, (groups, bnd, badcols) in enumerate(geo.blocks):
                S = Sts[bi]
                for sl, g in enumerate(groups):
                    rg = int(geo.rows[g])
                    S[sl * GSZ:(sl + 1) * GSZ, 1:1 + rg, :] = \
                        xb[b, g * GSZ:(g + 1) * GSZ,
                           int(geo.ry0[g]):int(geo.ry1[g]), :]
                nb2 = bnd + 2
                v1 = np.zeros((P, 1), np.float32)
                v0 = np.zeros((P, 1), np.float32)
                v2 = np.zeros((P, 1), np.float32)
                u0 = np.zeros((P, 1), np.float32)
                u1 = np.zeros((P, 1), np.float32)
                u2 = np.zeros((P, 1), np.float32)
                v0n = np.zeros((P, 1), np.float32)
                v2p = np.zeros((P, 1), np.float32)
                u0n = np.zeros((P, 1), np.float32)
                u2p = np.zeros((P, 1), np.float32)
                keepn = np.ones((P, 1), np.float32)
                keepp = np.ones((P, 1), np.float32)
                badmask = np.ones((P, len(badcols)), np.float32)
                for sl, g in enumerate(groups):
                    pp = slice(sl * GSZ, (sl + 1) * GSZ)
                    dxg = int(geo.dx[g])
                    v0[pp], v1[pp], v2[pp] = geo.v[g]
                    u0[pp], u1[pp], u2[pp] = geo.u[g]
                    if dxg < 0:
                        v0n[pp] = geo.v[g, 0]
                        u0n[pp] = geo.u[g, 0]
                        keepn[pp] = 0.0
                    if dxg > 0:
                        v2p[pp] = geo.v[g, 2]
                        u2p[pp] = geo.u[g, 2]
                        keepp[pp] = 0.0
                    mybad = (-dxg - 1) if dxg < 0 else (W - dxg)
                    for j, c in enumerate(badcols):
                        if dxg != 0 and mybad == c:
                            badmask[pp, j] = 0.0
                Sb = S.copy()
                for j, c in enumerate(badcols):
                    Sb[:, :, c] *= badmask[:, j:j + 1]
                T = np.zeros((P, bnd + 4, W), dtype=np.float32)
                T[:, 1:1 + nb2, :] = Sb[:, 0:nb2, :] * v1[:, :, None]
                T[:, 1:1 + nb2, 1:W] += Sb[:, 0:nb2, 0:W - 1] * v0[:, :, None]
                T[:, 1:1 + nb2, 0:W - 1] += Sb[:, 0:nb2, 1:W] * v2[:, :, None]
                T[:, 1:2 + nb2, 0:1] *= keepn[:, :, None]
                T[:, 2:2 + nb2, 0:1] += Sb[:, 0:nb2, W - 1:W] * v0n[:, :, None]
                T[:, 0:1 + nb2, W - 1:W] *= keepp[:, :, None]
                T[:, 0:nb2, W - 1:W] += Sb[:, 0:nb2, 0:1] * v2p[:, :, None]
                O = np.zeros((P, bnd + 4, W), dtype=np.float32)
                O[:, 1:1 + nb2, :] = T[:, 1:1 + nb2, :] * u1[:, :, None]
                O[:, 1:1 + nb2, :] += T[:, 0:nb2, :] * u0[:, :, None]
                O[:, 1:1 + nb2, :] += T[:, 2:2 + nb2, :] * u2[:, :, None]
                O[:, bnd + 3:bnd + 4, 0:1] = \
                    T[:, bnd + 2:bnd + 3, 0:1] * u0n[:, :, None]
                O[:, 0:1, W - 1:W] = T[:, 1:2, W - 1:W] * u2p[:, :, None]
                Of = O.reshape(P, -1)
                for sl, g in enumerate(groups):
                    vy0, vy1 = int(geo.vy0[g]), int(geo.vy1[g])
                    f0 = (vy0 - int(geo.dy[g]) - int(geo.ry0[g]) + 2) * W \
                        - int(geo.dx[g])
                    ln = (vy1 - vy0) * W
                    assert f0 >= 0 and f0 + ln <= Of.shape[1], (g, f0, ln)
                    ydev[b, g * GSZ:(g + 1) * GSZ, vy0:vy1, :] = \
                        Of[sl * GSZ:(sl + 1) * GSZ, f0:f0 + ln].reshape(
                            GSZ, vy1 - vy0, W)
        out_parts.append(ydev)
    return _assemble(geo, out_parts)


# revision 21
# speedup vs baseline: 1.2464x; 1.2464x over previous
"""Trainium2 Bass kernel for DisplaceChannel — conv-then-shift design.

Math (per channel c, group f = c // 16):
  off_px = offset[f] * 64; off_int = round(off_px); sub = off_px - off_int
  out = shift(depthwise 3x3 SAME conv of the window-masked image), where
  the window is the set of source pixels that survive the shift.  Key
  identity: shifting then convolving (reference order) equals convolving
  the *unshifted* window-masked image and shifting the result, because
  both reduce to the same zero-outside-window source semantics.

This removes all shift geometry from the compute and the DMA inner
dims:
  - Loads are per-group contiguous row bands of full-width rows
    (16 descriptors per group instead of ~16*rows).
  - Compute is a uniform separable 3-tap conv over [128, band, 64]
    tiles in bf16 (DVE 2-src ops run 2x at 16 bit).
  - Stores exploit flat addressing: the x-shift becomes a constant
    element offset into the flattened [rows, 64] plane, so each channel
    stores one contiguous run (16 descriptors per group).  Column wrap
    garbage lands only in output columns outside the valid window,
    which the host overwrites with zeros during reassembly.

Row masking: each group loads exactly its valid source rows into a
persistent per-block S tile whose margin rows are zeroed once at
startup and never rewritten.  Column masking: only the one column
adjacent to the window edge can pollute a valid output (conv reach is
1), so it is re-zeroed in Sb after each cast (small DVE memsets).

Engines: sync = HWDGE loads, gpsimd = cast fp32->bf16 + SWDGE
bf16->fp32 cast stores, ACT = center taps, DVE = four side taps +
boundary-column memsets.  Batch-parallel over 8 cores (2 batches per
core), identical SPMD program.
"""

import os
import sys
from collections import deque
from contextlib import ExitStack

import numpy as np

for _p in ("/opt/trn_rl_repo", "/root/.axon_site/_ro/trn_rl_repo"):
    if os.path.isdir(_p) and _p not in sys.path:
        sys.path.append(_p)

import concourse.bass as bass
import concourse.bacc as bacc
import concourse.mybir as mybir
import concourse.tile as tile
from concourse.bass_utils import run_bass_kernel_spmd

H = W = 64
C = 768
B = 16
N_CORES = 8
BPC = B // N_CORES          # batches per core
P = 128                     # partitions
NGRP = 48
GSZ = 16                    # channels per group
SCALE = 64.0
SIGMA = 0.5
FP32 = mybir.dt.float32
BF16 = mybir.dt.bfloat16
MULT = mybir.AluOpType.mult
ADD = mybir.AluOpType.add


def _geometry(offset: np.ndarray):
    """Integer shifts and separable 1-D taps per group, matching reference."""
    off_px = offset.astype(np.float32) * np.float32(SCALE)
    off_int = np.round(off_px)
    sub = off_px - off_int                      # [48, 2] (x, y)
    dx = off_int[:, 0].astype(np.int64)
    dy = off_int[:, 1].astype(np.int64)
    r = (np.arange(3, dtype=np.float32) - 1.0).astype(np.float32)
    ex = np.exp(-((r[None, :] + sub[:, 0:1]) ** 2) / (2.0 * SIGMA * SIGMA))
    ey = np.exp(-((r[None, :] + sub[:, 1:2]) ** 2) / (2.0 * SIGMA * SIGMA))
    v = ex / ex.sum(1, keepdims=True)           # [48, 3] horizontal taps
    u = ey / ey.sum(1, keepdims=True)           # [48, 3] vertical taps
    return dx, dy, v.astype(np.float32), u.astype(np.float32)


class _Geo:
    """Per-group window geometry + block partition."""

    def __init__(self, offset: np.ndarray):
        self.dx, self.dy, self.v, self.u = _geometry(offset)
        self.ry0 = np.maximum(0, -self.dy)
        self.ry1 = H - np.maximum(0, self.dy)
        self.cx0 = np.maximum(0, -self.dx)
        self.cx1 = W - np.maximum(0, self.dx)
        self.rows = self.ry1 - self.ry0          # valid source rows
        # valid dst windows (host reassembly)
        self.vy0 = np.maximum(0, self.ry0 - 1 + self.dy)
        self.vy1 = np.minimum(H, self.ry1 + 1 + self.dy)
        self.vx0 = np.maximum(0, self.cx0 - 1 + self.dx)
        self.vx1 = np.minimum(W, self.cx1 + 1 + self.dx)
        self.blocks = self._partition()

    def _partition(self):
        """Sort groups by band desc, split into consecutive runs of <=8
        minimizing sum of (block band + fixed overhead)."""
        order = sorted(range(NGRP), key=lambda g: -int(self.rows[g]))
        INF = float("inf")
        best = [INF] * (NGRP + 1)
        prev = [0] * (NGRP + 1)
        best[0] = 0.0
        for e in range(1, NGRP + 1):
            for s in range(max(0, e - 8), e):
                band = max(int(self.rows[order[i]]) for i in range(s, e))
                cost = best[s] + band + 10.0
                if cost < best[e]:
                    best[e] = cost
                    prev[e] = s
        cuts = []
        e = NGRP
        while e > 0:
            s = prev[e]
            cuts.append((s, e))
            e = s
        blocks = []
        for s, e in reversed(cuts):
            groups = [order[i] for i in range(s, e)]
            bnd = max(int(self.rows[g]) for g in groups)
            # distinct out-of-window boundary columns to zero in Sb
            badcols = sorted({
                (-int(self.dx[g]) - 1) if self.dx[g] < 0 else (W - int(self.dx[g]))
                for g in groups if self.dx[g] != 0})
            blocks.append((groups, bnd, badcols))
        return blocks


def _build(offset: np.ndarray) -> bass.Bass:
    geo = _Geo(offset)
    nblk = len(geo.blocks)

    # Per-block tap tables.  All compute ops run on the full 128
    # partitions (the ISA requires 32-aligned partition offsets), so
    # group-conditional ops use per-partition scalars that are zero (or
    # one, for the keep multiplies) on non-member partitions.
    # wc fp32 [P, 4]: v1, u1 (center taps), u0neg, u2pos (widening
    # corners).  ws fp32->bf16 [P, 16]: v0, v2, u0, u2 (side taps),
    # v0neg, v2pos (widening T taps), keep_neg, keep_pos, then up to 8
    # per-partition masks for the boundary-column zeroing.
    # single fp32 table [P, 20] per block:
    # 0:v1 1:v0 2:v2 3:u1 4:u0 5:u2 6:v0neg 7:v2pos 8:keep_neg
    # 9:keep_pos 10:u0neg 11:u2pos 12..19: boundary-column masks
    NW = 20
    ws = np.zeros((nblk, P, NW), dtype=np.float32)
    for bi, (groups, _, badcols) in enumerate(geo.blocks):
        assert len(badcols) <= 8
        for sl, g in enumerate(groups):
            pp = slice(sl * GSZ, (sl + 1) * GSZ)
            dxg = int(geo.dx[g])
            ws[bi, pp, 0] = geo.v[g, 1]
            ws[bi, pp, 1] = geo.v[g, 0]
            ws[bi, pp, 2] = geo.v[g, 2]
            ws[bi, pp, 3] = geo.u[g, 1]
            ws[bi, pp, 4] = geo.u[g, 0]
            ws[bi, pp, 5] = geo.u[g, 2]
            ws[bi, pp, 6] = geo.v[g, 0] if dxg < 0 else 0.0
            ws[bi, pp, 7] = geo.v[g, 2] if dxg > 0 else 0.0
            ws[bi, pp, 8] = 0.0 if dxg < 0 else 1.0
            ws[bi, pp, 9] = 0.0 if dxg > 0 else 1.0
            ws[bi, pp, 10] = geo.u[g, 0] if dxg < 0 else 0.0
            ws[bi, pp, 11] = geo.u[g, 2] if dxg > 0 else 0.0
            mybad = (-dxg - 1) if dxg < 0 else (W - dxg)
            for j, c in enumerate(badcols):
                ws[bi, pp, 12 + j] = 0.0 if (dxg != 0 and mybad == c) else 1.0

    nc = bacc.Bacc("TRN2", target_bir_lowering=False, debug=False)
    x_in = nc.dram_tensor("x", [BPC, C, H, W], FP32, kind="ExternalInput")
    y_out = nc.dram_tensor("y", [BPC, C, H, W], FP32, kind="ExternalOutput")
    ws_dram = nc.inline_tensor(ws, name="tapss")

    with tile.TileContext(nc) as tc, ExitStack() as ctx:
        w_pool = ctx.enter_context(tc.tile_pool(name="w", bufs=1))
        s_pool = ctx.enter_context(tc.tile_pool(name="s", bufs=1))
        o_pool = ctx.enter_context(tc.tile_pool(name="o", bufs=3))

        wsb = []
        for bi in range(nblk):
            f = w_pool.tile([P, NW], FP32, name=f"wf{bi}", tag=f"wf{bi}")
            nc.sync.dma_start(f[:], ws_dram[bi])
            wsb.append(f)
        t_pool = ctx.enter_context(tc.tile_pool(name="tmp", bufs=2))

        # Persistent per-block tiles.  S holds the loaded fp32 source
        # rows at local rows [1, 1+rows_g) with zero margins; Sb is its
        # bf16 copy; T is the horizontal-pass result with one extra
        # zero margin row on each side.
        St, Tt = [], []
        for bi, (groups, bnd, _) in enumerate(geo.blocks):
            S = s_pool.tile([P, bnd + 2, W], FP32, name=f"S{bi}", tag=f"S{bi}")
            T = s_pool.tile([P, bnd + 4, W], BF16, name=f"T{bi}", tag=f"T{bi}")
            St.append(S)
            Tt.append(T)

        inited = set()

        def emit_init(bi):
            # zero-margin invariants, established once per block (loads
            # rewrite only each group's interior rows afterwards).
            # Spread across engines so startup ramps fast.
            groups, bnd, _ = geo.blocks[bi]
            S, T = St[bi], Tt[bi]
            eng = (nc.vector, nc.gpsimd, nc.scalar)[bi % 3]
            if eng is nc.scalar:
                eng.memzero(S[:, :, :])
            else:
                eng.memset(S[:, :, :], 0.0)
            nc.gpsimd.memset(T[:, 0:bnd + 4:bnd + 3, :], 0.0)
            inited.add(bi)

        def emit_front(b, bi):
            if bi not in inited:
                emit_init(bi)
            groups, bnd, badcols = geo.blocks[bi]
            S, T = St[bi], Tt[bi]
            w = wsb[bi]
            nb2 = bnd + 2
            f = nb2 * W
            for sl, g in enumerate(groups):
                rg = int(geo.rows[g])
                nc.sync.dma_start(
                    S[sl * GSZ:(sl + 1) * GSZ, 1:1 + rg, :],
                    x_in[b, g * GSZ:(g + 1) * GSZ,
                         int(geo.ry0[g]):int(geo.ry1[g]), :])
            # zero the single source column adjacent to each window edge
            # (the only out-of-window column the conv can reach) via
            # per-partition 0/1 masks, directly on fp32 S.  On ACT so
            # the DVE never feeds the ACT h-mul stage (keeps the
            # two-stage pipeline decoupled).
            for j, c in enumerate(badcols):
                nc.scalar.mul(
                    S[:, 0:nb2, c:c + 1], S[:, 0:nb2, c:c + 1],
                    w[:, 12 + j:13 + j])
            # h-pass: ACT computes the three per-partition-scaled copies
            # (fp32 -> bf16), DVE combines with 2x tensor_tensor adds.
            Sf = S.rearrange("p r c -> p (r c)")
            Tf = T.rearrange("p r c -> p (r c)")
            Ta = t_pool.tile([P, nb2 * W], BF16, name="Ta", tag="Ta")
            Tb = t_pool.tile([P, nb2 * W], BF16, name="Tb", tag="Tb")
            nc.scalar.mul(Tf[:, W:W + f], Sf[:, 0:f], w[:, 0:1])
            nc.scalar.mul(Ta[:, 0:f], Sf[:, 0:f], w[:, 1:2])
            nc.scalar.mul(Tb[:, 0:f], Sf[:, 0:f], w[:, 2:3])
            nc.vector.tensor_tensor(
                T[:, 1:1 + nb2, 1:W], Ta.rearrange(
                    "p (r c) -> p r c", c=W)[:, 0:nb2, 0:W - 1],
                T[:, 1:1 + nb2, 1:W], ADD)
            nc.vector.tensor_tensor(
                T[:, 1:1 + nb2, 0:W - 1], Tb.rearrange(
                    "p (r c) -> p r c", c=W)[:, 0:nb2, 1:W],
                T[:, 1:1 + nb2, 0:W - 1], ADD)
            # widening columns (same masked keep/accumulate scheme)
            nc.vector.tensor_scalar_mul(
                T[:, 1:2 + nb2, 0:1], T[:, 1:2 + nb2, 0:1], w[:, 8:9])
            nc.vector.scalar_tensor_tensor(
                T[:, 2:2 + nb2, 0:1], S[:, 0:nb2, W - 1:W], w[:, 6:7],
                T[:, 2:2 + nb2, 0:1], MULT, ADD)
            nc.vector.tensor_scalar_mul(
                T[:, 0:1 + nb2, W - 1:W], T[:, 0:1 + nb2, W - 1:W],
                w[:, 9:10])
            nc.vector.scalar_tensor_tensor(
                T[:, 0:nb2, W - 1:W], S[:, 0:nb2, 0:1], w[:, 7:8],
                T[:, 0:nb2, W - 1:W], MULT, ADD)

        def emit_back(b, bi):
            groups, bnd, badcols = geo.blocks[bi]
            T = Tt[bi]
            w = wsb[bi]
            nb2 = bnd + 2
            f = nb2 * W
            Tf = T.rearrange("p r c -> p (r c)")
            O = o_pool.tile([P, bnd + 4, W], BF16, name="O", tag="O")
            nc.gpsimd.memset(O[:, 0:bnd + 4:bnd + 3, :], 0.0)
            Of = O.rearrange("p r c -> p (r c)")
            Oa = t_pool.tile([P, nb2 * W], BF16, name="Oa", tag="Oa")
            Ob = t_pool.tile([P, nb2 * W], BF16, name="Ob", tag="Ob")
            # v-pass: bf16 tensor_scalar premuls (4x) + tensor_tensor
            # adds (2x), all flat contiguous
            # widening-column corners first so ACT never waits on the
            # DVE v-chain (they touch only the edge rows)
            nc.scalar.mul(O[:, bnd + 3:bnd + 4, 0:1],
                          T[:, bnd + 2:bnd + 3, 0:1], w[:, 10:11])
            nc.scalar.mul(O[:, 0:1, W - 1:W],
                          T[:, 1:2, W - 1:W], w[:, 11:12])
            nc.vector.tensor_scalar_mul(Of[:, W:W + f], Tf[:, W:W + f],
                                        w[:, 3:4])
            nc.vector.tensor_scalar_mul(Oa[:, 0:f], Tf[:, 0:f], w[:, 4:5])
            nc.vector.tensor_tensor(Of[:, W:W + f], Oa[:, 0:f],
                                    Of[:, W:W + f], ADD)
            nc.vector.tensor_scalar_mul(Ob[:, 0:f], Tf[:, 2 * W:2 * W + f],
                                        w[:, 5:6])
            nc.vector.tensor_tensor(Of[:, W:W + f], Ob[:, 0:f],
                                    Of[:, W:W + f], ADD)
            for sl, g in enumerate(groups):
                vy0, vy1 = int(geo.vy0[g]), int(geo.vy1[g])
                f0 = (vy0 - int(geo.dy[g]) - int(geo.ry0[g]) + 2) * W \
                    - int(geo.dx[g])
                ln = (vy1 - vy0) * W
                nc.gpsimd.dma_start(
                    y_out[b, g * GSZ:(g + 1) * GSZ, vy0:vy1, :],
                    Of[sl * GSZ:(sl + 1) * GSZ, f0:f0 + ln])

        tiles = [(b, bi) for b in range(BPC) for bi in range(nblk)]
        pend = deque()
        DEPTH = 2
        for b, bi in tiles:
            emit_front(b, bi)
            pend.append((b, bi))
            if len(pend) > DEPTH:
                emit_back(*pend.popleft())
        while pend:
            emit_back(*pend.popleft())

    nc.compile()
    return nc


def _assemble(geo: _Geo, parts: list[np.ndarray]) -> np.ndarray:
    """Gather per-core device outputs into the full zero-padded result."""
    out = np.zeros((B, C, H, W), dtype=np.float32)
    for k, yk in enumerate(parts):
        for b in range(BPC):
            bb = k * BPC + b
            for g in range(NGRP):
                ch = slice(g * GSZ, (g + 1) * GSZ)
                vy0, vy1 = int(geo.vy0[g]), int(geo.vy1[g])
                vx0, vx1 = int(geo.vx0[g]), int(geo.vx1[g])
                out[bb, ch, vy0:vy1, vx0:vx1] = \
                    yk[b, ch, vy0:vy1, vx0:vx1]
    return out


def _run(x: np.ndarray, offset: np.ndarray, trace: bool = False):
    x = np.ascontiguousarray(x, dtype=np.float32)
    offset = np.ascontiguousarray(offset, dtype=np.float32)
    geo = _Geo(offset)
    nc = _build(offset)
    in_maps = [
        {"x": x[k * BPC:(k + 1) * BPC]} for k in range(N_CORES)
    ]
    res = run_bass_kernel_spmd(
        nc, in_maps, core_ids=list(range(N_CORES)), trace=trace
    )
    out = _assemble(geo, [res.results[k]["y"] for k in range(N_CORES)])
    return out, res


def kernel(x: np.ndarray, offset: np.ndarray) -> np.ndarray:
    return _run(x, offset)[0]


def _numpy_sim(x: np.ndarray, offset: np.ndarray) -> np.ndarray:
    """Pure-numpy emulation of the exact device dataflow (fp32, no bf16
    rounding) for index validation."""
    geo = _Geo(offset)
    out_parts = []
    for k in range(N_CORES):
        xb = x[k * BPC:(k + 1) * BPC]
        ydev = np.full((BPC, C, H, W), np.nan, dtype=np.float32)
        Sts = {}
        for bi, (groups, bnd, _) in enumerate(geo.blocks):
            Sts[bi] = np.zeros((P, bnd + 2, W), dtype=np.float32)
        for b in range(BPC):
            for bi, (groups, bnd, badcols) in enumerate(geo.blocks):
                S = Sts[bi]
                for sl, g in enumerate(groups):
                    rg = int(geo.rows[g])
                    S[sl * GSZ:(sl + 1) * GSZ, 1:1 + rg, :] = \
                        xb[b, g * GSZ:(g + 1) * GSZ,
                           int(geo.ry0[g]):int(geo.ry1[g]), :]
                nb2 = bnd + 2
                v1 = np.zeros((P, 1), np.float32)
                v0 = np.zeros((P, 1), np.float32)
                v2 = np.zeros((P, 1), np.float32)
                u0 = np.zeros((P, 1), np.float32)
                u1 = np.zeros((P, 1), np.float32)
                u2 = np.zeros((P, 1), np.float32)
                v0n = np.zeros((P, 1), np.float32)
                v2p = np.zeros((P, 1), np.float32)
                u0n = np.zeros((P, 1), np.float32)
                u2p = np.zeros((P, 1), np.float32)
                keepn = np.ones((P, 1), np.float32)
                keepp = np.ones((P, 1), np.float32)
                badmask = np.ones((P, len(badcols)), np.float32)
                for sl, g in enumerate(groups):
                    pp = slice(sl * GSZ, (sl + 1) * GSZ)
                    dxg = int(geo.dx[g])
                    v0[pp], v1[pp], v2[pp] = geo.v[g]
                    u0[pp], u1[pp], u2[pp] = geo.u[g]
                    if dxg < 0:
                        v0n[pp] = geo.v[g, 0]
                        u0n[pp] = geo.u[g, 0]
                        keepn[pp] = 0.0
                    if dxg > 0:
                        v2p[pp] = geo.v[g, 2]
                        u2p[pp] = geo.u[g, 2]
                        keepp[pp] = 0.0
                    mybad = (-dxg - 1) if dxg < 0 else (W - dxg)
                    for j, c in enumerate(badcols):
                        if dxg != 0 and mybad == c:
                            badmask[pp, j] = 0.0
                Sb = S.copy()
                for j, c in enumerate(badcols):
                    Sb[:, :, c] *= badmask[:, j:j + 1]
                T = np.zeros((P, bnd + 4, W), dtype=np.float32)
                T[:, 1:1 + nb2, :] = Sb[:, 0:nb2, :] * v1[:, :, None]
                T[:, 1:1 + nb2, 1:W] += Sb[:, 0:nb2, 0:W - 1] * v0[:, :, None]
                T[:, 1:1 + nb2, 0:W - 1] += Sb[:, 0:nb2, 1:W] * v2[:, :, None]
                T[:, 1:2 + nb2, 0:1] *= keepn[:, :, None]
                T[:, 2:2 + nb2, 0:1] += Sb[:, 0:nb2, W - 1:W] * v0n[:, :, None]
                T[:, 0:1 + nb2, W - 1:W] *= keepp[:, :, None]
                T[:, 0:nb2, W - 1:W] += Sb[:, 0:nb2, 0:1] * v2p[:, :, None]
                O = np.zeros((P, bnd + 4, W), dtype=np.float32)
                O[:, 1:1 + nb2, :] = T[:, 1:1 + nb2, :] * u1[:, :, None]
                O[:, 1:1 + nb2, :] += T[:, 0:nb2, :] * u0[:, :, None]
                O[:, 1:1 + nb2, :] += T[:, 2:2 + nb2, :] * u2[:, :, None]
                O[:, bnd + 3:bnd + 4, 0:1] = \
                    T[:, bnd + 2:bnd + 3, 0:1] * u0n[:, :, None]
                O[:, 0:1, W - 1:W] = T[:, 1:2, W - 1:W] * u2p[:, :, None]
                Of = O.reshape(P, -1)
                for sl, g in enumerate(groups):
                    vy0, vy1 = int(geo.vy0[g]), int(geo.vy1[g])
                    f0 = (vy0 - int(geo.dy[g]) - int(geo.ry0[g]) + 2) * W \
                        - int(geo.dx[g])
                    ln = (vy1 - vy0) * W
                    assert f0 >= 0 and f0 + ln <= Of.shape[1], (g, f0, ln)
                    ydev[b, g * GSZ:(g + 1) * GSZ, vy0:vy1, :] = \
                        Of[sl * GSZ:(sl + 1) * GSZ, f0:f0 + ln].reshape(
                            GSZ, vy1 - vy0, W)
        out_parts.append(ydev)
    return _assemble(geo, out_parts)


# revision 22
# speedup vs baseline: 1.2572x; 1.0087x over previous
"""Trainium2 Bass kernel for DisplaceChannel — conv-then-shift design.

Math (per channel c, group f = c // 16):
  off_px = offset[f] * 64; off_int = round(off_px); sub = off_px - off_int
  out = shift(depthwise 3x3 SAME conv of the window-masked image), where
  the window is the set of source pixels that survive the shift.  Key
  identity: shifting then convolving (reference order) equals convolving
  the *unshifted* window-masked image and shifting the result, because
  both reduce to the same zero-outside-window source semantics.

This removes all shift geometry from the compute and the DMA inner
dims:
  - Loads are per-group contiguous row bands of full-width rows
    (16 descriptors per group instead of ~16*rows).
  - Compute is a uniform separable 3-tap conv over [128, band, 64]
    tiles in bf16 (DVE 2-src ops run 2x at 16 bit).
  - Stores exploit flat addressing: the x-shift becomes a constant
    element offset into the flattened [rows, 64] plane, so each channel
    stores one contiguous run (16 descriptors per group).  Column wrap
    garbage lands only in output columns outside the valid window,
    which the host overwrites with zeros during reassembly.

Row masking: each group loads exactly its valid source rows into a
persistent per-block S tile whose margin rows are zeroed once at
startup and never rewritten.  Column masking: only the one column
adjacent to the window edge can pollute a valid output (conv reach is
1), so it is re-zeroed in Sb after each cast (small DVE memsets).

Engines: sync = HWDGE loads, gpsimd = cast fp32->bf16 + SWDGE
bf16->fp32 cast stores, ACT = center taps, DVE = four side taps +
boundary-column memsets.  Batch-parallel over 8 cores (2 batches per
core), identical SPMD program.
"""

import os
import sys
from collections import deque
from contextlib import ExitStack

import numpy as np

for _p in ("/opt/trn_rl_repo", "/root/.axon_site/_ro/trn_rl_repo"):
    if os.path.isdir(_p) and _p not in sys.path:
        sys.path.append(_p)

import concourse.bass as bass
import concourse.bacc as bacc
import concourse.mybir as mybir
import concourse.tile as tile
from concourse.bass_utils import run_bass_kernel_spmd

H = W = 64
C = 768
B = 16
N_CORES = 8
BPC = B // N_CORES          # batches per core
P = 128                     # partitions
NGRP = 48
GSZ = 16                    # channels per group
SCALE = 64.0
SIGMA = 0.5
FP32 = mybir.dt.float32
BF16 = mybir.dt.bfloat16
MULT = mybir.AluOpType.mult
ADD = mybir.AluOpType.add


def _geometry(offset: np.ndarray):
    """Integer shifts and separable 1-D taps per group, matching reference."""
    off_px = offset.astype(np.float32) * np.float32(SCALE)
    off_int = np.round(off_px)
    sub = off_px - off_int                      # [48, 2] (x, y)
    dx = off_int[:, 0].astype(np.int64)
    dy = off_int[:, 1].astype(np.int64)
    r = (np.arange(3, dtype=np.float32) - 1.0).astype(np.float32)
    ex = np.exp(-((r[None, :] + sub[:, 0:1]) ** 2) / (2.0 * SIGMA * SIGMA))
    ey = np.exp(-((r[None, :] + sub[:, 1:2]) ** 2) / (2.0 * SIGMA * SIGMA))
    v = ex / ex.sum(1, keepdims=True)           # [48, 3] horizontal taps
    u = ey / ey.sum(1, keepdims=True)           # [48, 3] vertical taps
    return dx, dy, v.astype(np.float32), u.astype(np.float32)


class _Geo:
    """Per-group window geometry + block partition."""

    def __init__(self, offset: np.ndarray):
        self.dx, self.dy, self.v, self.u = _geometry(offset)
        self.ry0 = np.maximum(0, -self.dy)
        self.ry1 = H - np.maximum(0, self.dy)
        self.cx0 = np.maximum(0, -self.dx)
        self.cx1 = W - np.maximum(0, self.dx)
        self.rows = self.ry1 - self.ry0          # valid source rows
        # valid dst windows (host reassembly)
        self.vy0 = np.maximum(0, self.ry0 - 1 + self.dy)
        self.vy1 = np.minimum(H, self.ry1 + 1 + self.dy)
        self.vx0 = np.maximum(0, self.cx0 - 1 + self.dx)
        self.vx1 = np.minimum(W, self.cx1 + 1 + self.dx)
        self.blocks = self._partition()

    def _partition(self):
        """Sort groups by band desc, split into consecutive runs of <=8
        minimizing sum of (block band + fixed overhead)."""
        order = sorted(range(NGRP), key=lambda g: -int(self.rows[g]))
        INF = float("inf")
        best = [INF] * (NGRP + 1)
        prev = [0] * (NGRP + 1)
        best[0] = 0.0
        for e in range(1, NGRP + 1):
            for s in range(max(0, e - 8), e):
                band = max(int(self.rows[order[i]]) for i in range(s, e))
                cost = best[s] + band + 10.0
                if cost < best[e]:
                    best[e] = cost
                    prev[e] = s
        cuts = []
        e = NGRP
        while e > 0:
            s = prev[e]
            cuts.append((s, e))
            e = s
        blocks = []
        for s, e in reversed(cuts):
            groups = [order[i] for i in range(s, e)]
            bnd = max(int(self.rows[g]) for g in groups)
            # distinct out-of-window boundary columns to zero in Sb
            badcols = sorted({
                (-int(self.dx[g]) - 1) if self.dx[g] < 0 else (W - int(self.dx[g]))
                for g in groups if self.dx[g] != 0})
            blocks.append((groups, bnd, badcols))
        return blocks


def _build(offset: np.ndarray) -> bass.Bass:
    geo = _Geo(offset)
    nblk = len(geo.blocks)

    # Per-block tap tables.  All compute ops run on the full 128
    # partitions (the ISA requires 32-aligned partition offsets), so
    # group-conditional ops use per-partition scalars that are zero (or
    # one, for the keep multiplies) on non-member partitions.
    # wc fp32 [P, 4]: v1, u1 (center taps), u0neg, u2pos (widening
    # corners).  ws fp32->bf16 [P, 16]: v0, v2, u0, u2 (side taps),
    # v0neg, v2pos (widening T taps), keep_neg, keep_pos, then up to 8
    # per-partition masks for the boundary-column zeroing.
    # single fp32 table [P, 20] per block:
    # 0:v1 1:v0 2:v2 3:u1 4:u0 5:u2 6:v0neg 7:v2pos 8:keep_neg
    # 9:keep_pos 10:u0neg 11:u2pos 12..19: boundary-column masks
    NW = 20
    ws = np.zeros((nblk, P, NW), dtype=np.float32)
    for bi, (groups, _, badcols) in enumerate(geo.blocks):
        assert len(badcols) <= 8
        for sl, g in enumerate(groups):
            pp = slice(sl * GSZ, (sl + 1) * GSZ)
            dxg = int(geo.dx[g])
            ws[bi, pp, 0] = geo.v[g, 1]
            ws[bi, pp, 1] = geo.v[g, 0]
            ws[bi, pp, 2] = geo.v[g, 2]
            ws[bi, pp, 3] = geo.u[g, 1]
            ws[bi, pp, 4] = geo.u[g, 0]
            ws[bi, pp, 5] = geo.u[g, 2]
            ws[bi, pp, 6] = geo.v[g, 0] if dxg < 0 else 0.0
            ws[bi, pp, 7] = geo.v[g, 2] if dxg > 0 else 0.0
            ws[bi, pp, 8] = 0.0 if dxg < 0 else 1.0
            ws[bi, pp, 9] = 0.0 if dxg > 0 else 1.0
            ws[bi, pp, 10] = geo.u[g, 0] if dxg < 0 else 0.0
            ws[bi, pp, 11] = geo.u[g, 2] if dxg > 0 else 0.0
            mybad = (-dxg - 1) if dxg < 0 else (W - dxg)
            for j, c in enumerate(badcols):
                ws[bi, pp, 12 + j] = 0.0 if (dxg != 0 and mybad == c) else 1.0

    nc = bacc.Bacc("TRN2", target_bir_lowering=False, debug=False)
    x_in = nc.dram_tensor("x", [BPC, C, H, W], FP32, kind="ExternalInput")
    y_out = nc.dram_tensor("y", [BPC, C, H, W], FP32, kind="ExternalOutput")
    ws_dram = nc.inline_tensor(ws, name="tapss")

    with tile.TileContext(nc) as tc, ExitStack() as ctx:
        w_pool = ctx.enter_context(tc.tile_pool(name="w", bufs=1))
        s_pool = ctx.enter_context(tc.tile_pool(name="s", bufs=1))
        o_pool = ctx.enter_context(tc.tile_pool(name="o", bufs=3))

        wsb = []
        for bi in range(nblk):
            f = w_pool.tile([P, NW], FP32, name=f"wf{bi}", tag=f"wf{bi}")
            nc.sync.dma_start(f[:], ws_dram[bi])
            wsb.append(f)
        t_pool = ctx.enter_context(tc.tile_pool(name="tmp", bufs=2))

        # Persistent per-block tiles.  S holds the loaded fp32 source
        # rows at local rows [1, 1+rows_g) with zero margins; Sb is its
        # bf16 copy; T is the horizontal-pass result with one extra
        # zero margin row on each side.
        St, Tt = [], []
        for bi, (groups, bnd, _) in enumerate(geo.blocks):
            S = s_pool.tile([P, bnd + 2, W], FP32, name=f"S{bi}", tag=f"S{bi}")
            T = s_pool.tile([P, bnd + 4, W], BF16, name=f"T{bi}", tag=f"T{bi}")
            St.append(S)
            Tt.append(T)

        inited = set()

        def emit_init(bi):
            # zero-margin invariants, established once per block (loads
            # rewrite only each group's interior rows afterwards).
            # Spread across engines so startup ramps fast.
            groups, bnd, _ = geo.blocks[bi]
            S, T = St[bi], Tt[bi]
            eng = (nc.vector, nc.gpsimd, nc.scalar)[bi % 3]
            if eng is nc.scalar:
                eng.memzero(S[:, :, :])
            else:
                eng.memset(S[:, :, :], 0.0)
            nc.gpsimd.memset(T[:, 0:bnd + 4:bnd + 3, :], 0.0)
            inited.add(bi)

        def emit_front(b, bi):
            if bi not in inited:
                emit_init(bi)
            groups, bnd, badcols = geo.blocks[bi]
            S, T = St[bi], Tt[bi]
            w = wsb[bi]
            nb2 = bnd + 2
            f = nb2 * W
            for sl, g in enumerate(groups):
                rg = int(geo.rows[g])
                nc.sync.dma_start(
                    S[sl * GSZ:(sl + 1) * GSZ, 1:1 + rg, :],
                    x_in[b, g * GSZ:(g + 1) * GSZ,
                         int(geo.ry0[g]):int(geo.ry1[g]), :])
            # zero the single source column adjacent to each window edge
            # (the only out-of-window column the conv can reach) via
            # per-partition 0/1 masks, directly on fp32 S.  On ACT so
            # the DVE never feeds the ACT h-mul stage (keeps the
            # two-stage pipeline decoupled).
            for j, c in enumerate(badcols):
                nc.scalar.mul(
                    S[:, 0:nb2, c:c + 1], S[:, 0:nb2, c:c + 1],
                    w[:, 12 + j:13 + j])
            # h-pass: ACT computes the three per-partition-scaled copies
            # (fp32 -> bf16), DVE combines with 2x tensor_tensor adds.
            Sf = S.rearrange("p r c -> p (r c)")
            Tf = T.rearrange("p r c -> p (r c)")
            Ta = t_pool.tile([P, nb2 * W], BF16, name="Ta", tag="Ta")
            Tb = t_pool.tile([P, nb2 * W], BF16, name="Tb", tag="Tb")
            nc.scalar.mul(Tf[:, W:W + f], Sf[:, 0:f], w[:, 0:1])
            nc.scalar.mul(Ta[:, 0:f], Sf[:, 0:f], w[:, 1:2])
            nc.scalar.mul(Tb[:, 0:f], Sf[:, 0:f], w[:, 2:3])
            nc.vector.tensor_tensor(
                T[:, 1:1 + nb2, 1:W], Ta.rearrange(
                    "p (r c) -> p r c", c=W)[:, 0:nb2, 0:W - 1],
                T[:, 1:1 + nb2, 1:W], ADD)
            nc.vector.tensor_tensor(
                T[:, 1:1 + nb2, 0:W - 1], Tb.rearrange(
                    "p (r c) -> p r c", c=W)[:, 0:nb2, 1:W],
                T[:, 1:1 + nb2, 0:W - 1], ADD)
            # widening columns (same masked keep/accumulate scheme)
            nc.vector.tensor_scalar_mul(
                T[:, 1:2 + nb2, 0:1], T[:, 1:2 + nb2, 0:1], w[:, 8:9])
            nc.vector.scalar_tensor_tensor(
                T[:, 2:2 + nb2, 0:1], S[:, 0:nb2, W - 1:W], w[:, 6:7],
                T[:, 2:2 + nb2, 0:1], MULT, ADD)
            nc.vector.tensor_scalar_mul(
                T[:, 0:1 + nb2, W - 1:W], T[:, 0:1 + nb2, W - 1:W],
                w[:, 9:10])
            nc.vector.scalar_tensor_tensor(
                T[:, 0:nb2, W - 1:W], S[:, 0:nb2, 0:1], w[:, 7:8],
                T[:, 0:nb2, W - 1:W], MULT, ADD)

        def emit_back(b, bi):
            groups, bnd, badcols = geo.blocks[bi]
            T = Tt[bi]
            w = wsb[bi]
            nb2 = bnd + 2
            f = nb2 * W
            Tf = T.rearrange("p r c -> p (r c)")
            O = o_pool.tile([P, bnd + 4, W], BF16, name="O", tag="O")
            nc.gpsimd.memset(O[:, 0:bnd + 4:bnd + 3, :], 0.0)
            Of = O.rearrange("p r c -> p (r c)")
            Oa = t_pool.tile([P, nb2 * W], BF16, name="Oa", tag="Oa")
            Ob = t_pool.tile([P, nb2 * W], BF16, name="Ob", tag="Ob")
            # v-pass: bf16 tensor_scalar premuls (4x) + tensor_tensor
            # adds (2x), all flat contiguous
            # widening-column corners first so ACT never waits on the
            # DVE v-chain (they touch only the edge rows)
            nc.scalar.mul(O[:, bnd + 3:bnd + 4, 0:1],
                          T[:, bnd + 2:bnd + 3, 0:1], w[:, 10:11])
            nc.scalar.mul(O[:, 0:1, W - 1:W],
                          T[:, 1:2, W - 1:W], w[:, 11:12])
            nc.vector.tensor_scalar_mul(Of[:, W:W + f], Tf[:, W:W + f],
                                        w[:, 3:4])
            nc.vector.tensor_scalar_mul(Oa[:, 0:f], Tf[:, 0:f], w[:, 4:5])
            nc.vector.tensor_tensor(Of[:, W:W + f], Oa[:, 0:f],
                                    Of[:, W:W + f], ADD)
            nc.vector.tensor_scalar_mul(Ob[:, 0:f], Tf[:, 2 * W:2 * W + f],
                                        w[:, 5:6])
            nc.vector.tensor_tensor(Of[:, W:W + f], Ob[:, 0:f],
                                    Of[:, W:W + f], ADD)
            for sl, g in enumerate(groups):
                vy0, vy1 = int(geo.vy0[g]), int(geo.vy1[g])
                f0 = (vy0 - int(geo.dy[g]) - int(geo.ry0[g]) + 2) * W \
                    - int(geo.dx[g])
                ln = (vy1 - vy0) * W
                nc.gpsimd.dma_start(
                    y_out[b, g * GSZ:(g + 1) * GSZ, vy0:vy1, :],
                    Of[sl * GSZ:(sl + 1) * GSZ, f0:f0 + ln])

        tiles = [(b, bi) for b in range(BPC) for bi in range(nblk)]
        pend = deque()
        DEPTH = 1
        for b, bi in tiles:
            if len(pend) >= DEPTH + 1:
                emit_back(*pend.popleft())
            emit_front(b, bi)
            pend.append((b, bi))
        while pend:
            emit_back(*pend.popleft())

    nc.compile()
    return nc


def _assemble(geo: _Geo, parts: list[np.ndarray]) -> np.ndarray:
    """Gather per-core device outputs into the full zero-padded result."""
    out = np.zeros((B, C, H, W), dtype=np.float32)
    for k, yk in enumerate(parts):
        for b in range(BPC):
            bb = k * BPC + b
            for g in range(NGRP):
                ch = slice(g * GSZ, (g + 1) * GSZ)
                vy0, vy1 = int(geo.vy0[g]), int(geo.vy1[g])
                vx0, vx1 = int(geo.vx0[g]), int(geo.vx1[g])
                out[bb, ch, vy0:vy1, vx0:vx1] = \
                    yk[b, ch, vy0:vy1, vx0:vx1]
    return out


def _run(x: np.ndarray, offset: np.ndarray, trace: bool = False):
    x = np.ascontiguousarray(x, dtype=np.float32)
    offset = np.ascontiguousarray(offset, dtype=np.float32)
    geo = _Geo(offset)
    nc = _build(offset)
    in_maps = [
        {"x": x[k * BPC:(k + 1) * BPC]} for k in range(N_CORES)
    ]
    res = run_bass_kernel_spmd(
        nc, in_maps, core_ids=list(range(N_CORES)), trace=trace
    )
    out = _assemble(geo, [res.results[k]["y"] for k in range(N_CORES)])
    return out, res


def kernel(x: np.ndarray, offset: np.ndarray) -> np.ndarray:
    return _run(x, offset)[0]


def _numpy_sim(x: np.ndarray, offset: np.ndarray) -> np.ndarray:
    """Pure-numpy emulation of the exact device dataflow (fp32, no bf16
    rounding) for index validation."""
    geo = _Geo(offset)
    out_parts = []
    for k in range(N_CORES):
        xb = x[k * BPC:(k + 1) * BPC]
        ydev = np.full((BPC, C, H, W), np.nan, dtype=np.float32)
        Sts = {}
        for bi, (groups, bnd, _) in enumerate(geo.blocks):
            Sts[bi] = np.zeros((P, bnd + 2, W), dtype=np.float32)
        for b in range(BPC):
            for bi, (groups, bnd, badcols) in enumerate(geo.blocks):
                S = Sts[bi]
                for sl, g in enumerate(groups):
                    rg = int(geo.rows[g])
                    S[sl * GSZ:(sl + 1) * GSZ, 1:1 + rg, :] = \
                        xb[b, g * GSZ:(g + 1) * GSZ,
                           int(geo.ry0[g]):int(geo.ry1[g]), :]
                nb2 = bnd + 2
                v1 = np.zeros((P, 1), np.float32)
                v0 = np.zeros((P, 1), np.float32)
                v2 = np.zeros((P, 1), np.float32)
                u0 = np.zeros((P, 1), np.float32)
                u1 = np.zeros((P, 1), np.float32)
                u2 = np.zeros((P, 1), np.float32)
                v0n = np.zeros((P, 1), np.float32)
                v2p = np.zeros((P, 1), np.float32)
                u0n = np.zeros((P, 1), np.float32)
                u2p = np.zeros((P, 1), np.float32)
                keepn = np.ones((P, 1), np.float32)
                keepp = np.ones((P, 1), np.float32)
                badmask = np.ones((P, len(badcols)), np.float32)
                for sl, g in enumerate(groups):
                    pp = slice(sl * GSZ, (sl + 1) * GSZ)
                    dxg = int(geo.dx[g])
                    v0[pp], v1[pp], v2[pp] = geo.v[g]
                    u0[pp], u1[pp], u2[pp] = geo.u[g]
                    if dxg < 0:
                        v0n[pp] = geo.v[g, 0]
                        u0n[pp] = geo.u[g, 0]
                        keepn[pp] = 0.0
                    if dxg > 0:
                        v2p[pp] = geo.v[g, 2]
                        u2p[pp] = geo.u[g, 2]
                        keepp[pp] = 0.0
                    mybad = (-dxg - 1) if dxg < 0 else (W - dxg)
                    for j, c in enumerate(badcols):
                        if dxg != 0 and mybad == c:
                            badmask[pp, j] = 0.0
                Sb = S.copy()
                for j, c in enumerate(badcols):
                    Sb[:, :, c] *= badmask[:, j:j + 1]
                T = np.zeros((P, bnd + 4, W), dtype=np.float32)
                T[:, 1:1 + nb2, :] = Sb[:, 0:nb2, :] * v1[:, :, None]
                T[:, 1:1 + nb2, 1:W] += Sb[:, 0:nb2, 0:W - 1] * v0[:, :, None]
                T[:, 1:1 + nb2, 0:W - 1] += Sb[:, 0:nb2, 1:W] * v2[:, :, None]
                T[:, 1:2 + nb2, 0:1] *= keepn[:, :, None]
                T[:, 2:2 + nb2, 0:1] += Sb[:, 0:nb2, W - 1:W] * v0n[:, :, None]
                T[:, 0:1 + nb2, W - 1:W] *= keepp[:, :, None]
                T[:, 0:nb2, W - 1:W] += Sb[:, 0:nb2, 0:1] * v2p[:, :, None]
                O = np.zeros((P, bnd + 4, W), dtype=np.float32)
                O[:, 1:1 + nb2, :] = T[:, 1:1 + nb2, :] * u1[:, :, None]
                O[:, 1:1 + nb2, :] += T[:, 0:nb2, :] * u0[:, :, None]
                O[:, 1:1 + nb2, :] += T[:, 2:2 + nb2, :] * u2[:, :, None]
                O[:, bnd + 3:bnd + 4, 0:1] = \
                    T[:, bnd + 2:bnd + 3, 0:1] * u0n[:, :, None]
                O[:, 0:1, W - 1:W] = T[:, 1:2, W - 1:W] * u2p[:, :, None]
                Of = O.reshape(P, -1)
                for sl, g in enumerate(groups):
                    vy0, vy1 = int(geo.vy0[g]), int(geo.vy1[g])
                    f0 = (vy0 - int(geo.dy[g]) - int(geo.ry0[g]) + 2) * W \
                        - int(geo.dx[g])
                    ln = (vy1 - vy0) * W
                    assert f0 >= 0 and f0 + ln <= Of.shape[1], (g, f0, ln)
                    ydev[b, g * GSZ:(g + 1) * GSZ, vy0:vy1, :] = \
                        Of[sl * GSZ:(sl + 1) * GSZ, f0:f0 + ln].reshape(
                            GSZ, vy1 - vy0, W)
        out_parts.append(ydev)
    return _assemble(geo, out_parts)
